# revision 2
# baseline (speedup 1.0000x reference)
"""Distributed DMPNN (2-layer GRU message passing) for 8 TRN2 NeuronCores.

Math identity exploited (linearity of segment_sum):
    msg  = concat(x[src], ea) @ Wm^T = y[src] + ea @ We^T,  y = x @ Wx^T
    agg  = seg_sum(msg, dst) = seg_sum(y[src], dst) + A @ We^T,
           A = seg_sum(ea, dst)  (layer independent)

Sharding: edges sorted by dst; core c owns dst in [c*NPC, (c+1)*NPC).
Each core computes agg and the GRU update for its own node range only.
The single collective is an AllGather of y2 = x1 @ Wx2^T between layers.

Per-core device pipeline:
  P1: y1 = x @ Wx1^T for ALL nodes (replicated; x^T is a per-core input)
      -> y1_dram (bf16 rows, 256B each)
  P2: per dst-tile t (128 dst nodes):
      - dma_gather y1 rows for the tile's padded edge slots
      - build one-hot(dst) via iota==dstf (DVE), bf16
      - PE scatter matmuls: psum[dst,0:128] += onehot^T @ y1_gath
                            psum[dst,128:192] += onehot^T @ ea
      - A^T tile via PE transpose; + A@We1^T into psum[:,0:128]
      - GRU1 matmuls + elementwise -> x1 (node major) + x1^T (bf16)
      - y2 tile = x1 @ Wx2^T -> y2_local dram
  P3: AllGather y2_local -> y2_full
  P4: per dst-tile t: gather y2, scatter (aggT orientation), + We2-term
      via A^T, GRU2 -> out rows.
"""

import numpy as np

import concourse.bass as bass
import concourse.mybir as mybir
import concourse.bacc as bacc
import concourse.tile as tile
from concourse.masks import make_identity

F32 = mybir.dt.float32
BF16 = mybir.dt.bfloat16
I16 = mybir.dt.int16
NPBF16 = np.dtype(mybir.dt.np(BF16))

N_CORES = 8
P = 128


# ---------------------------------------------------------------- host side
def preprocess(x, edge_index, edge_attr,
               W_msg1, Wih1, Whh1, bih1, bhh1,
               W_msg2, Wih2, Whh2, bih2, bhh2):
    """Host-side integer/layout preprocessing. Returns (meta, in_maps)."""
    x = np.asarray(x, np.float32)
    n_nodes, hidden = x.shape
    edge_dim = edge_attr.shape[1]
    assert hidden == 128 and edge_dim == 64
    assert n_nodes % N_CORES == 0
    npc = n_nodes // N_CORES          # nodes per core
    nt = (npc + P - 1) // P           # dst tiles per core
    nta = (n_nodes + P - 1) // P      # tiles over all nodes

    src = np.asarray(edge_index[0], np.int64)
    dst = np.asarray(edge_index[1], np.int64)
    ea = np.asarray(edge_attr, np.float32)

    # Balance 64-node windows: place nodes round-robin by in-degree so each
    # window's edge count is near-uniform (shrinks the uniform chunk padding
    # and balances cores). new_of_old maps old node id -> new position.
    W0 = 64
    n_windows = n_nodes // W0
    deg = np.bincount(dst, minlength=n_nodes)
    by_deg = np.argsort(-deg, kind="stable")
    new_of_old = np.empty(n_nodes, np.int64)
    # serpentine round-robin over windows to even out systematic bias
    slot_in_w = np.zeros(n_windows, np.int64)
    widx = np.concatenate([np.arange(n_windows), np.arange(n_windows)[::-1]])
    wseq = np.tile(widx, (n_nodes // (2 * n_windows)) + 1)[:n_nodes]
    for i, node in enumerate(by_deg):
        wnd = wseq[i]
        new_of_old[node] = wnd * W0 + slot_in_w[wnd]
        slot_in_w[wnd] += 1
    assert (slot_in_w == W0).all()
    old_of_new = np.empty(n_nodes, np.int64)
    old_of_new[new_of_old] = np.arange(n_nodes)

    x = x[old_of_new]                 # x in NEW node order
    src = new_of_old[src]
    dst = new_of_old[dst]

    order = np.lexsort((src, dst))
    src_s, dst_s = src[order], dst[order]
    ea_s = ea[order]

    # per (core, tile) edge counts -> uniform K_CHUNKS
    tile_of = dst_s // P              # global dst tile id (core*nt + t) iff npc%P==0
    # dst tile within core must be based on core-local index:
    core_of = dst_s // npc
    dloc = dst_s - core_of * npc
    t_of = dloc // P
    counts = np.zeros((N_CORES, nt), np.int64)
    np.add.at(counts, (core_of, t_of), 1)
    k_chunks = int(np.max((counts + P - 1) // P))
    slots_per_tile = k_chunks * P
    slots = nt * slots_per_tile

    meta = dict(n_nodes=n_nodes, npc=npc, nt=nt, nta=nta, k_chunks=k_chunks,
                slots=slots)

    # slot assignment per core
    in_maps = []
    # boundaries of (core, tile) groups in the sorted edge array
    grp = core_of * nt + t_of
    # edges are sorted by dst so grp is nondecreasing
    starts = np.searchsorted(grp, np.arange(N_CORES * nt), side="left")
    ends = np.searchsorted(grp, np.arange(N_CORES * nt), side="right")

    bz = not (np.any(bih1) or np.any(bhh1) or np.any(bih2) or np.any(bhh2))
    meta["biases_zero"] = bz
    assert bz, "nonzero biases not implemented in kernel yet"

    xT = np.zeros((P, nta * P), np.float32)
    xT[:, :n_nodes] = x.T
    xT = xT.astype(NPBF16)

    for c in range(N_CORES):
        g_idx = np.zeros(slots, np.int64)
        dst_f = np.full(slots, -1.0, np.float32)
        ea_arr = np.zeros((slots, edge_dim), np.float32)
        for t in range(nt):
            g = c * nt + t
            s0, s1 = starts[g], ends[g]
            cnt = s1 - s0
            base = t * slots_per_tile
            g_idx[base:base + cnt] = src_s[s0:s1]
            dst_f[base:base + cnt] = (dloc[s0:s1] - t * P).astype(np.float32)
            ea_arr[base:base + cnt] = ea_s[s0:s1]

        # wrapped int16 gather indices, per tile: [16, k_chunks*8] blocks,
        # replicated to all 8 Q7-core partition stripes -> [128, slots//16]
        gidx_w = np.zeros((16, slots // 16), np.int16)
        for t in range(nt):
            blk = g_idx[t * slots_per_tile:(t + 1) * slots_per_tile]
            gidx_w[:, t * (slots_per_tile // 16):(t + 1) * (slots_per_tile // 16)] = \
                blk.reshape(-1, 16).T.astype(np.int16)
        gidx_w = np.tile(gidx_w, (8, 1))

        # dstf [128, nt*k_chunks] (partition = lane within chunk)
        dstf = dst_f.reshape(nt * k_chunks, P).T.astype(NPBF16).copy()
        # ea arranged [128, nt*k_chunks*64]
        ea_in = np.ascontiguousarray(
            ea_arr.reshape(nt * k_chunks, P, edge_dim).transpose(1, 0, 2)
        ).reshape(P, nt * k_chunks * edge_dim).astype(NPBF16)

        # node-major x shard (f32) [128, nt*128]
        xs = np.zeros((P, nt * P), np.float32)
        rows = x[c * npc:(c + 1) * npc]  # [npc, 128]
        xsr = np.zeros((nt * P, P), np.float32)
        xsr[:npc] = rows
        xs = np.ascontiguousarray(
            xsr.reshape(nt, P, P).transpose(1, 0, 2)).reshape(P, nt * P)
        # transposed x shard (bf16) [128, nt*128]
        xsT = np.zeros((P, nt * P), np.float32)
        xsT[:, :npc] = rows.T
        xsT = xsT.astype(NPBF16)

        in_maps.append({
            "xT": xT,
            "xsT": xsT,
            "xs": xs.astype(np.float32),
            "gidx": gidx_w,
            "dstf": dstf,
            "ea": ea_in,
            "wx1r": np.ascontiguousarray(W_msg1[:, :128].T).astype(NPBF16),
            "we1r": np.ascontiguousarray(W_msg1[:, 128:].T).astype(NPBF16),
            "wih1t": np.ascontiguousarray(np.asarray(Wih1).T).astype(NPBF16),
            "whh1t": np.ascontiguousarray(np.asarray(Whh1).T).astype(NPBF16),
            "wx2r": np.ascontiguousarray(W_msg2[:, :128].T).astype(NPBF16),
            "we2r": np.ascontiguousarray(W_msg2[:, 128:].T).astype(NPBF16),
            "wih2t": np.ascontiguousarray(np.asarray(Wih2).T).astype(NPBF16),
            "whh2t": np.ascontiguousarray(np.asarray(Whh2).T).astype(NPBF16),
        })
    return meta, in_maps


# ---------------------------------------------------------------- device side
def build(meta, n_iters=1):
    n_nodes = meta["n_nodes"]
    npc, nt, nta, K = meta["npc"], meta["nt"], meta["nta"], meta["k_chunks"]
    spt = K * P  # slots per tile

    nc = bacc.Bacc("TRN2", target_bir_lowering=False, debug=False,
                   num_devices=N_CORES)

    xT_d = nc.dram_tensor("xT", [P, nta * P], BF16, kind="ExternalInput")
    xsT_d = nc.dram_tensor("xsT", [P, nt * P], BF16, kind="ExternalInput")
    xs_d = nc.dram_tensor("xs", [P, nt * P], F32, kind="ExternalInput")
    gidx_d = nc.dram_tensor("gidx", [P, nt * spt // 16], I16, kind="ExternalInput")
    dstf_d = nc.dram_tensor("dstf", [P, nt * K], BF16, kind="ExternalInput")
    ea_d = nc.dram_tensor("ea", [P, nt * K * 64], BF16, kind="ExternalInput")
    w_d = {}
    for nm, shape in [("wx1r", [P, P]), ("we1r", [64, P]),
                      ("wih1t", [P, 384]), ("whh1t", [P, 384]),
                      ("wx2r", [P, P]), ("we2r", [64, P]),
                      ("wih2t", [P, 384]), ("whh2t", [P, 384])]:
        w_d[nm] = nc.dram_tensor(nm, shape, BF16, kind="ExternalInput")
    out_d = nc.dram_tensor("out", [npc, P], F32, kind="ExternalOutput")

    with tile.TileContext(nc) as tc:
        with tc.tile_pool(name="persist", bufs=1) as pp, \
             tc.tile_pool(name="work", bufs=3) as wp, \
             tc.tile_pool(name="small", bufs=3) as sp, \
             tc.tile_pool(name="psA", bufs=2, space="PSUM") as ppsA, \
             tc.tile_pool(name="psB", bufs=2, space="PSUM") as ppsB, \
             tc.tile_pool(name="psC", bufs=2, space="PSUM") as ppsC, \
             tc.tile_pool(name="dram", bufs=1, space="DRAM") as dp:

            # ---- persistent SBUF state
            xT = pp.tile([P, nta * P], BF16, tag="xT")
            nc.sync.dma_start(xT[:], xT_d[:])
            xsT = pp.tile([P, nt * P], BF16, tag="xsT")
            nc.sync.dma_start(xsT[:], xsT_d[:])
            xs = pp.tile([P, nt * P], F32, tag="xs")
            nc.sync.dma_start(xs[:], xs_d[:])
            gidx = pp.tile([P, nt * spt // 16], I16, tag="gidx")
            nc.sync.dma_start(gidx[:], gidx_d[:])
            dstf = pp.tile([P, nt * K], BF16, tag="dstf")
            nc.sync.dma_start(dstf[:], dstf_d[:])
            w = {}
            for nm, h in w_d.items():
                w[nm] = pp.tile(list(h.shape), BF16, tag=nm)
                nc.sync.dma_start(w[nm][:], h[:])

            ident = pp.tile([P, P], BF16, tag="ident")
            make_identity(nc, ident[:])
            iota_i = pp.tile([P, 1, P], I16, tag="iota_i")
            nc.gpsimd.iota(iota_i[:], pattern=[[0, 1], [1, P]], base=0,
                           channel_multiplier=0)
            iota_b = pp.tile([P, 1, P], BF16, tag="iota_b")
            nc.vector.tensor_copy(iota_b[:], iota_i[:])

            x1_nm = pp.tile([P, nt * P], F32, tag="x1_nm")
            x1T = pp.tile([P, nt * P], BF16, tag="x1T")
            AT = pp.tile([64, nt * P], BF16, tag="AT")

            # ---- DRAM scratch
            y1_dram = dp.tile([nta * P, P], BF16)
            y2_loc = dp.tile([npc, P], BF16)
            y2_full = dp.tile([n_nodes, P], BF16, addr_space="Shared")

            for _ in range(n_iters):
                # ============================ P1: y1 for all nodes
                GROUP = 16  # node tiles per staging buffer
                for g0 in range(0, nta, GROUP):
                    g1 = min(g0 + GROUP, nta)
                    stage = wp.tile([P, GROUP * P], BF16, tag="y1stage")
                    for ta in range(g0, g1):
                        psy = ppsC.tile([P, P], F32, tag="psy")
                        nc.tensor.matmul(psy[:], lhsT=xT[:, ta * P:(ta + 1) * P],
                                         rhs=w["wx1r"][:], start=True, stop=True)
                        nc.vector.tensor_copy(stage[:, (ta - g0) * P:(ta - g0 + 1) * P],
                                              psy[:])
                    dst_ap = y1_dram[g0 * P:g1 * P, :].rearrange(
                        "(ta p) f -> p ta f", p=P)
                    nc.sync.dma_start(dst_ap, stage[:, :(g1 - g0) * P].rearrange(
                        "p (ta f) -> p ta f", f=P))

                # ============================ P2: layer 1 per dst tile
                for t in range(nt):
                    rows = min(P, npc - t * P)
                    yg = wp.tile([P, K, P], BF16, tag="yg")
                    nc.gpsimd.dma_gather(
                        yg[:], y1_dram[:], gidx[:, t * (spt // 16):(t + 1) * (spt // 16)],
                        spt, spt, P)
                    eat = wp.tile([P, K * 64], BF16, tag="eat")
                    nc.sync.dma_start(eat[:], ea_d[:, t * K * 64:(t + 1) * K * 64])
                    oh = wp.tile([P, K, P], BF16, tag="oh")
                    nc.vector.tensor_tensor(
                        out=oh[:],
                        in0=iota_b[:].to_broadcast([P, K, P]),
                        in1=dstf[:, t * K:(t + 1) * K]
                            .rearrange("p (c o) -> p c o", o=1)
                            .to_broadcast([P, K, P]),
                        op=mybir.AluOpType.is_equal)

                    ps = ppsA.tile([P, 192], F32, tag="scat")
                    for k in range(K):
                        nc.tensor.matmul(ps[:, 0:128], lhsT=oh[:, k, :],
                                         rhs=yg[:, k, :],
                                         start=(k == 0), stop=False,
                                         skip_group_check=True)
                        nc.tensor.matmul(ps[:, 128:192], lhsT=oh[:, k, :],
                                         rhs=eat[:, k * 64:(k + 1) * 64],
                                         start=(k == 0), stop=(k == K - 1),
                                         skip_group_check=True)
                    # A^T tile
                    a_nm = sp.tile([P, 64], BF16, tag="a_nm")
                    nc.vector.tensor_copy(a_nm[:], ps[:, 128:192])
                    pst = ppsB.tile([64, P], BF16, tag="pst")
                    nc.tensor.transpose(pst[:], a_nm[:], ident[:])
                    nc.vector.tensor_copy(AT[:, t * P:(t + 1) * P], pst[:])
                    nc.tensor.matmul(ps[:, 0:128], lhsT=AT[:, t * P:(t + 1) * P],
                                     rhs=w["we1r"][:], start=False, stop=True,
                                     skip_group_check=True)
                    # agg -> aggT
                    agg_nm = sp.tile([P, P], BF16, tag="agg_nm")
                    nc.vector.tensor_copy(agg_nm[:], ps[:, 0:128])
                    pst2 = ppsB.tile([P, P], BF16, tag="pst2")
                    nc.tensor.transpose(pst2[:], agg_nm[:], ident[:])
                    aggT = sp.tile([P, P], BF16, tag="aggT")
                    nc.vector.tensor_copy(aggT[:], pst2[:])

                    _gru_tile(nc, ppsC, sp, t, aggT, xsT[:, t * P:(t + 1) * P],
                              w["wih1t"], w["whh1t"], xs[:, t * P:(t + 1) * P],
                              x1_nm[:, t * P:(t + 1) * P])
                    # x1T tile
                    h1b = sp.tile([P, P], BF16, tag="h1b")
                    nc.vector.tensor_copy(h1b[:], x1_nm[:, t * P:(t + 1) * P])
                    pst3 = ppsB.tile([P, P], BF16, tag="pst2")
                    nc.tensor.transpose(pst3[:], h1b[:], ident[:])
                    nc.vector.tensor_copy(x1T[:, t * P:(t + 1) * P], pst3[:])
                    # y2 tile
                    psy2 = ppsC.tile([P, P], F32, tag="psy")
                    nc.tensor.matmul(psy2[:], lhsT=x1T[:, t * P:(t + 1) * P],
                                     rhs=w["wx2r"][:], start=True, stop=True)
                    y2b = sp.tile([P, P], BF16, tag="y2b")
                    nc.vector.tensor_copy(y2b[:], psy2[:])
                    nc.sync.dma_start(y2_loc[t * P:t * P + rows, :], y2b[0:rows, :])

                # ============================ P3: AllGather y2
                nc.gpsimd.collective_compute(
                    "AllGather", mybir.AluOpType.bypass,
                    replica_groups=[list(range(N_CORES))],
                    ins=[y2_loc[:].opt()], outs=[y2_full[:].opt()])

                # ============================ P4: layer 2 per dst tile
                for t in range(nt):
                    rows = min(P, npc - t * P)
                    yg = wp.tile([P, K, P], BF16, tag="yg")
                    nc.gpsimd.dma_gather(
                        yg[:], y2_full[:], gidx[:, t * (spt // 16):(t + 1) * (spt // 16)],
                        spt, spt, P)
                    oh = wp.tile([P, K, P], BF16, tag="oh")
                    nc.vector.tensor_tensor(
                        out=oh[:],
                        in0=iota_b[:].to_broadcast([P, K, P]),
                        in1=dstf[:, t * K:(t + 1) * K]
                            .rearrange("p (c o) -> p c o", o=1)
                            .to_broadcast([P, K, P]),
                        op=mybir.AluOpType.is_equal)

                    psT = ppsA.tile([P, P], F32, tag="scat")  # aggT2 directly
                    for k in range(K):
                        nc.tensor.matmul(psT[:], lhsT=yg[:, k, :], rhs=oh[:, k, :],
                                         start=(k == 0), stop=False,
                                         skip_group_check=True)
                    nc.tensor.matmul(psT[:], lhsT=w["we2r"][:],
                                     rhs=AT[:, t * P:(t + 1) * P],
                                     start=False, stop=True, skip_group_check=True)
                    aggT2 = sp.tile([P, P], BF16, tag="aggT")
                    nc.vector.tensor_copy(aggT2[:], psT[:])

                    h2 = sp.tile([P, P], F32, tag="h2")
                    _gru_tile(nc, ppsC, sp, t, aggT2, x1T[:, t * P:(t + 1) * P],
                              w["wih2t"], w["whh2t"], x1_nm[:, t * P:(t + 1) * P],
                              h2[:])
                    nc.sync.dma_start(out_d[t * P:t * P + rows, :], h2[0:rows, :])

    nc.compile()
    return nc


def _gru_tile(nc, psp, sp, t, mT, hT, wihT, whhT, h_nm, out_nm):
    """GRU cell for one 128-node tile, node-major output.

    mT: [128(feat), 128(node)] bf16 (aggregated message, transposed)
    hT: [128(feat), 128(node)] bf16 (hidden state, transposed)
    h_nm: [128(node), 128(feat)] f32 (hidden, node major)
    out_nm: [128(node), 128(feat)] f32 target
    """
    g = psp.tile([P, 512], F32, tag="gru")
    # rz = gi_rz + gh_rz
    nc.tensor.matmul(g[:, 0:256], lhsT=mT[:], rhs=wihT[:, 0:256],
                     start=True, stop=False, skip_group_check=True)
    nc.tensor.matmul(g[:, 0:256], lhsT=hT[:], rhs=whhT[:, 0:256],
                     start=False, stop=True, skip_group_check=True)
    nc.tensor.matmul(g[:, 256:384], lhsT=mT[:], rhs=wihT[:, 256:384],
                     start=True, stop=True, skip_group_check=True)
    nc.tensor.matmul(g[:, 384:512], lhsT=hT[:], rhs=whhT[:, 256:384],
                     start=True, stop=True, skip_group_check=True)
    rz = sp.tile([P, 256], F32, tag="rz")
    nc.scalar.activation(rz[:], g[:, 0:256],
                         mybir.ActivationFunctionType.Sigmoid)
    tmp = sp.tile([P, P], F32, tag="gtmp")
    nc.vector.tensor_mul(tmp[:], rz[:, 0:128], g[:, 384:512])
    nc.vector.tensor_add(tmp[:], tmp[:], g[:, 256:384])
    n_t = sp.tile([P, P], F32, tag="gn")
    nc.scalar.activation(n_t[:], tmp[:], mybir.ActivationFunctionType.Tanh)
    d_t = sp.tile([P, P], F32, tag="gd")
    nc.vector.tensor_sub(d_t[:], h_nm, n_t[:])
    nc.vector.tensor_mul(d_t[:], rz[:, 128:256], d_t[:])
    nc.vector.tensor_add(out_nm, n_t[:], d_t[:])



# ---------------------------------------------------------------- entry point
_CACHE = {}


def kernel(**inputs) -> np.ndarray:
    """Full (unsharded) inputs in, full [20000, 128] float32 output out.

    Shards edges by destination node across 8 NeuronCores, compiles the
    Bass kernel (cached across calls for identical graph shape), executes
    via run_bass_kernel_spmd, and concatenates the per-core node shards.
    """
    from concourse import bass_utils

    meta, in_maps = preprocess(**inputs)
    key = (meta["n_nodes"], meta["npc"], meta["nt"], meta["k_chunks"])
    nc = _CACHE.get(key)
    if nc is None:
        nc = build(meta)
        _CACHE[key] = nc
    res = bass_utils.run_bass_kernel_spmd(nc, in_maps,
                                          core_ids=list(range(N_CORES)))
    out = np.concatenate([res.results[c]["out"] for c in range(N_CORES)],
                         axis=0)
    out = out[meta["new_of_old"]]     # back to original node order
    return np.ascontiguousarray(out, dtype=np.float32)
